# revision 45
# baseline (speedup 1.0000x reference)
# MoE (8 experts, top-2) on 8 TRN2 NeuronCores — hidden-dim sharded with a
# mixed bf16/fp8 precision schedule.
#
# Every core owns a 512-wide slice of the hidden dim of ALL 8 experts and
# processes ALL token-expert pairs: per-core PE work is independent of
# routing balance (no capacity padding). Host sums the 8 partial outputs.
#
# Precision schedule: pairs whose gate value is below a threshold (chosen so
# the fp8-assigned pairs hold F8_MASS of total gate^2 mass, simulated and
# hw-verified rel err 1.83e-2 < 2e-2 gate) run both matmuls in fp8 e4m3 with DoubleRow
# perf mode: 2 contraction chunks (256) per instruction at the same 216 ns
# as a bf16 128-chunk instruction — measured true 2x. W1 is pre-scaled by
# 32 and W2 by 64 so their sigma~1 values use the e4m3 range; the 1/32 is
# folded into the gelu activation's scale and the 1/64 into the host gate
# multiply.
#
# Emission is software-pipelined one block deep (mm1 of block i+1 before
# mm2 of block i) so the gelu latency of the last h-chunk never stalls PE.
#
# Shapes hardcoded for B=4, S=2048, D=1024, H=4096, E=8; program built per
# routing split and cached on the per-expert (n_bf16, n_fp8) tuple.

import numpy as np
import ml_dtypes

NUM_EXPERTS = 8
TOP_K = 2
P = 128          # SBUF partitions
TB = 512         # token block (matmul moving free size)
D = 1024
H = 4096
HS = H // NUM_EXPERTS   # hidden slice per core (512)
KD = D // P      # mm1 contraction chunks (8) / mm2 output row chunks
MT = HS // P     # mm1 output m-tiles per expert segment (4)
KH2 = HS // P    # mm2 contraction chunks (4)
S1 = 32.0        # fp8 W1 pre-scale
S2 = 64.0        # fp8 W2 pre-scale
F8_MASS = 0.11   # fraction of gate^2 mass allowed onto the fp8 path
                 # (simulated rel err 1.83e-2 vs the 2e-2 gate; the device
                 # error matches the numpy simulation to 4+ digits and the
                 # error model is routing-independent, so this is safe)

_program_cache = {}


def _seg_blocks(n):
    rem = n % TB
    return [TB] * (n // TB) + ([rem] if rem else [])


def _block_schedule(segs):
    # shared by host packing and program builder — layouts must agree.
    # The very first block is carved down to 128 tokens so PE starts as
    # soon as b1 + one W1 m-slice + a small x block have landed.
    blocks = []  # (e, is_f8, t0, src0, tbs, first_of_seg)
    t0 = 0
    src_bf = 0
    src_f8 = 0
    for e in range(len(segs)):
        n_bf, n_f8 = segs[e]
        first = True
        bsizes = _seg_blocks(n_bf)
        if e == 0 and bsizes and bsizes[0] == TB:
            # ramp-up: PE starts after ~1 MB; block 0 is long enough to
            # keep the mm1/ACT pipeline busy while DMA is still ramping
            bsizes = [384, TB - 384] + bsizes[1:]
        for tbs in bsizes:
            blocks.append((e, False, t0, src_bf, tbs, first))
            first = False
            t0 += tbs
            src_bf += tbs
        for tbs in _seg_blocks(n_f8):
            blocks.append((e, True, t0, src_f8, tbs, first))
            first = False
            t0 += tbs
            src_f8 += tbs
    return blocks


def _x_pairs(blocks):
    # fetch two consecutive same-dtype blocks with one DMA; blocks 0/1 stay
    # single for the startup-critical chain.  Shared by host packing and
    # builder: paired blocks are packed as one [_, ta+tb] unit.
    pairs = {}
    j = 2
    while j + 1 < len(blocks):
        if blocks[j][1] == blocks[j + 1][1]:
            pairs[j] = j + 1
            j += 2
        else:
            j += 1
    return pairs


def _build_program(segs):
    # segs: tuple of (n_bf, n_f8) per expert; n_f8 multiples of 16
    import concourse.mybir as mybir
    import concourse.tile as tile
    from concourse import bacc

    bf = mybir.dt.bfloat16
    f8 = mybir.dt.float8e4
    f32 = mybir.dt.float32
    Gelu = mybir.ActivationFunctionType.Gelu_apprx_tanh
    DR = mybir.MatmulPerfMode.DoubleRow

    E = NUM_EXPERTS
    Ntot = sum(a + b for a, b in segs)
    Nbf = sum(a for a, _ in segs)
    Nf8 = sum(b for _, b in segs)

    nc = bacc.Bacc(None, target_bir_lowering=False, debug=False)
    xtb = nc.declare_dram_parameter("xtb", [P, max(1, KD * Nbf)], bf, isOutput=False).ap()
    xt8 = nc.declare_dram_parameter("xt8", [P, max(1, KD * Nf8)], f8, isOutput=False).ap()
    w1 = nc.declare_dram_parameter("w1", [P, E, KD, HS], bf, isOutput=False).ap()
    w2 = nc.declare_dram_parameter("w2", [P, E, KH2, D], bf, isOutput=False).ap()
    # w18 (KD/2*2*HS = 4096) and w28 (KH2/2*2*D = 4096) packed per expert
    wf8 = nc.declare_dram_parameter("wf8", [P, E, 8192], f8, isOutput=False).ap()
    b1t = nc.declare_dram_parameter("b1t", [P, E * MT], f32, isOutput=False).ap()
    # block-packed output: block at t0 occupies [KD*t0, KD*(t0+tbs)) per
    # partition, contiguous — 8x fewer DMA descriptor runs than [D, Ntot]
    ytr = nc.declare_dram_parameter("ytr", [P, KD * Ntot], bf, isOutput=True).ap()

    # block schedule: per expert, bf16 blocks then fp8 blocks
    blocks = _block_schedule(segs)

    with tile.TileContext(nc) as tc:
        with (
            tc.tile_pool(name="w1p", bufs=2) as w1p,
            tc.tile_pool(name="w2p", bufs=2) as w2p,
            tc.tile_pool(name="w18p", bufs=2) as w18p,
            tc.tile_pool(name="w28p", bufs=2) as w28p,
            tc.tile_pool(name="cst", bufs=1) as cst,
            tc.tile_pool(name="xin", bufs=3) as xpool,
            tc.tile_pool(name="hbuf", bufs=2) as hpool,
            tc.tile_pool(name="yout", bufs=3) as ypool,
            tc.tile_pool(name="ph", bufs=4, space="PSUM") as php,
            tc.tile_pool(name="py", bufs=4, space="PSUM") as pyp,
        ):
            b1_sb = cst.tile([P, E * MT], f32, tag="b1sb")
            nc.sync.dma_start(b1_sb, b1t)

            w1_sb = [None] * E
            w2_sb = [None] * E
            w18_sb = [None] * E
            w28_sb = [None] * E

            def fetch_w1(e, only_m0=False, skip_m0=False):
                # expert 0 is fetched as 4 m-slice tiles so the first matmul
                # group only waits for 256 KB (startup-critical); later
                # experts fetch one contiguous 1 MB DMA (fewer descriptors)
                if only_m0 or skip_m0:
                    if not skip_m0:
                        w1_sb[e] = []
                    for m in range(MT):
                        if (only_m0 and m > 0) or (skip_m0 and m == 0):
                            continue
                        t = w1p.tile([P, KD, P], bf, tag=f"w1sb{m}",
                                     name=f"w1sb{e}_{m}")
                        nc.sync.dma_start(t, w1[:, e, :, m * P:(m + 1) * P])
                        w1_sb[e].append(t)
                else:
                    t = w1p.tile([P, KD, HS], bf, tag="w1full",
                                 name=f"w1sb{e}")
                    nc.sync.dma_start(t, w1[:, e, :, :])
                    w1_sb[e] = t

            def fetch_w2(e):
                w2_sb[e] = w2p.tile([P, KH2, D], bf, tag="w2sb", name=f"w2sb{e}")
                nc.sync.dma_start(w2_sb[e], w2[:, e, :, :])

            def fetch_f8w(e):
                t = w18p.tile([P, 8192], f8, tag="wf8sb", name=f"wf8sb{e}")
                nc.sync.dma_start(t, wf8[:, e, :])
                w18_sb[e] = t[:, :4096].rearrange(
                    "p (k j m) -> p k j m", k=KD // 2, j=2)
                w28_sb[e] = t[:, 4096:].rearrange(
                    "p (k j m) -> p k j m", k=KH2 // 2, j=2)

            fetch_w1(0, only_m0=True)  # startup chain: b1 + w1[0]m0 + x0

            # fetch plan: expert e's weights stream during segment e-1,
            # spread one ~1 MB fetch per block so the x stream is never
            # starved behind a weight burst (x starvation stalls PE and
            # triggers HAM clock-throttle).  Fetches are emitted after that
            # iteration's pending mm2 (pool-buffer WAR ordering).
            seg_start = {}
            for j, bj in enumerate(blocks):
                seg_start.setdefault(bj[0], j)
            fetch_plan = {}

            def plan(i, kind, fe):
                fetch_plan.setdefault(i, []).append((kind, fe))

            plan(0, "w2", 0)  # needed by mm2(block 0), emitted at i==1
            plan(2, "f8", 0)
            for fe in range(1, E):
                anchor = seg_start[fe - 1] + (3 if fe == 1 else 0)
                last = seg_start[fe] - 1  # stay within segment fe-1
                plan(min(anchor, last), "w1", fe)
                plan(min(anchor + 1, last), "w2", fe)
                plan(min(anchor + 2, last), "f8", fe)

            x_pair_next = _x_pairs(blocks)
            x_ap = {}  # block index -> AP for its x data

            def emit_x_single(blk):
                e, is_f8, t0, src0, tbs, first = blk
                if is_f8:
                    xt_blk = xpool.tile([P, KD // 2, 2, tbs], f8, tag="xt8")
                    nc.sync.dma_start(
                        xt_blk,
                        xt8[:, KD * src0:KD * (src0 + tbs)].rearrange(
                            "p (k j c) -> p k j c", k=KD // 2, j=2
                        ),
                    )
                else:
                    xt_blk = xpool.tile([P, KD, tbs], bf, tag="xtb")
                    nc.sync.dma_start(
                        xt_blk,
                        xtb[:, KD * src0:KD * (src0 + tbs)].rearrange(
                            "p (k c) -> p k c", k=KD
                        ),
                    )
                return xt_blk

            def emit_x(i):
                if i in x_ap:
                    return x_ap.pop(i)
                jj = x_pair_next.get(i)
                if jj is None:
                    return emit_x_single(blocks[i])
                blk_a, blk_b = blocks[i], blocks[jj]
                ta, tb = blk_a[4], blk_b[4]
                is_f8 = blk_a[1]
                src0 = blk_a[3]
                if is_f8:
                    tile_ = xpool.tile([P, KD // 2, 2, ta + tb], f8, tag="xt8")
                    nc.sync.dma_start(
                        tile_,
                        xt8[:, KD * src0:KD * (src0 + ta + tb)].rearrange(
                            "p (k j c) -> p k j c", k=KD // 2, j=2
                        ),
                    )
                    x_ap[jj] = tile_[:, :, :, ta:]
                    return tile_[:, :, :, :ta]
                tile_ = xpool.tile([P, KD, ta + tb], bf, tag="xtb")
                nc.sync.dma_start(
                    tile_,
                    xtb[:, KD * src0:KD * (src0 + ta + tb)].rearrange(
                        "p (k c) -> p k c", k=KD
                    ),
                )
                x_ap[jj] = tile_[:, :, ta:]
                return tile_[:, :, :ta]

            def emit_mm1(blk, xt_blk):
                e, is_f8, t0, src0, tbs, first = blk
                if is_f8:
                    hT = hpool.tile([P, KH2, tbs], f8, tag="hT8")
                    for m in range(MT):
                        ph = php.tile([P, tbs], f32, tag="ph")
                        for k in range(KD // 2):
                            nc.tensor.matmul(
                                ph,
                                w18_sb[e][:, k, :, m * P:(m + 1) * P],
                                xt_blk[:, k, :, :],
                                start=(k == 0),
                                stop=(k == KD // 2 - 1),
                                perf_mode=DR,
                            )
                        nc.scalar.activation(
                            hT[:, m, :], ph, Gelu,
                            bias=b1_sb[:, e * MT + m:e * MT + m + 1],
                            scale=1.0 / S1,
                        )
                else:
                    hT = hpool.tile([P, KH2, tbs], bf, tag="hT")
                    for m in range(MT):
                        ph = php.tile([P, tbs], f32, tag="ph")
                        w1e = w1_sb[e]
                        for k in range(KD):
                            nc.tensor.matmul(
                                ph,
                                w1e[m][:, k, :] if isinstance(w1e, list)
                                else w1e[:, k, m * P:(m + 1) * P],
                                xt_blk[:, k, :],
                                start=(k == 0),
                                stop=(k == KD - 1),
                            )
                        nc.scalar.activation(
                            hT[:, m, :], ph, Gelu,
                            bias=b1_sb[:, e * MT + m:e * MT + m + 1],
                        )
                return hT

            def emit_mm2(blk, hT, pair=None):
                # pair=(tile, off, total, flush): even blocks allocate a
                # 2-block tile; odd blocks reuse it and flush one DMA
                e, is_f8, t0, src0, tbs, first = blk
                if pair is None:
                    yt_all = ypool.tile([P, KD * tbs], bf, tag="yt")
                    off, total, flush, dma_t0 = 0, KD * tbs, True, t0
                else:
                    yt_all, off, total, flush, dma_t0 = pair
                for dd in range(KD):
                    py = pyp.tile([P, tbs], f32, tag="py")
                    if is_f8:
                        for k in range(KH2 // 2):
                            nc.tensor.matmul(
                                py,
                                w28_sb[e][:, k, :, dd * P:(dd + 1) * P],
                                hT[:, 2 * k:2 * k + 2, :],
                                start=(k == 0),
                                stop=(k == KH2 // 2 - 1),
                                perf_mode=DR,
                            )
                    else:
                        for k in range(KH2):
                            nc.tensor.matmul(
                                py,
                                w2_sb[e][:, k, dd * P:(dd + 1) * P],
                                hT[:, k, :],
                                start=(k == 0),
                                stop=(k == KH2 - 1),
                            )
                    # alternate evacuation between DVE and ACT so the PSUM
                    # bank drain never trails the matmul groups
                    dst = yt_all[:, off + dd * tbs:off + (dd + 1) * tbs]
                    if dd % 2 == 0:
                        nc.vector.tensor_copy(dst, py)
                    else:
                        nc.scalar.copy(dst, py)
                if flush:
                    nc.sync.dma_start(
                        ytr[:, KD * dma_t0:KD * dma_t0 + total],
                        yt_all[:, :total],
                    )

            pending = None  # (blk, hT) awaiting mm2
            x_next = None  # x(block 1), primed ahead of the w2[0] fetch
            ypair = None   # open 2-block y tile: (tile, off, total, flush, t0)
            nmm2 = 0       # mm2 emission count (pairing parity)

            def emit_mm2_paired(blk, hT):
                nonlocal ypair, nmm2
                tbs = blk[4]
                if ypair is None:
                    if nmm2 + 1 < len(blocks):
                        nxt = blocks[nmm2 + 1][4]
                        tile_ = ypool.tile([P, KD * (tbs + nxt)], bf, tag="yt")
                        emit_mm2(blk, hT,
                                 pair=(tile_, 0, KD * tbs, False, blk[2]))
                        ypair = (tile_, KD * tbs, KD * (tbs + nxt), True,
                                 blk[2])
                    else:
                        emit_mm2(blk, hT)  # unpaired tail block
                else:
                    emit_mm2(blk, hT, pair=ypair)
                    ypair = None
                nmm2 += 1

            for i, blk in enumerate(blocks):
                e, is_f8, t0, src0, tbs, first = blk
                xt_blk = x_next if x_next is not None else emit_x(i)
                x_next = None
                if i == 0:
                    fetch_w1(0, skip_m0=True)  # behind x0 in the queue
                hT = emit_mm1(blk, xt_blk)
                if i == 0 and len(blocks) > 1:
                    x_next = emit_x(1)
                # weight streaming, ordered behind the x blocks that are
                # needed sooner (DMAs drain roughly in emission order):
                # i==1: w2[0]; i==2: fp8 weights[0]; i==3: all of seg 1;
                # then one segment ahead at each later segment boundary.
                # Startup fetches use fresh pool buffers so they may precede
                # the pending mm2; the segment-boundary fetch reuses a buffer
                # whose last reader is the pending mm2, so it must follow it.
                if pending is not None:
                    emit_mm2_paired(*pending)
                for kind, fe in fetch_plan.get(i, ()):
                    (fetch_w1 if kind == "w1" else
                     fetch_w2 if kind == "w2" else fetch_f8w)(fe)
                pending = (blk, hT)
            emit_mm2_paired(*pending)
    nc.compile()
    return nc


def _ensure_trace_hooks():
    # bass_utils' trace path (taken when BASS_TRACE=1 is set externally)
    # imports antenv.axon_hooks, which this image lacks. Shim it (and the
    # artifact upload, which needs a bucket) only when missing, so tracing
    # degrades gracefully instead of crashing.
    import sys
    import types

    try:
        import antenv.axon_hooks  # noqa: F401
        return
    except ImportError:
        pass
    try:
        import antenv

        mod = types.ModuleType("antenv.axon_hooks")
        state = {"hook": None}
        mod.set_axon_ntff_profile_hook = lambda h: state.__setitem__("hook", h)
        mod.get_axon_ntff_profile_hook = lambda: state["hook"]
        sys.modules["antenv.axon_hooks"] = mod
        antenv.axon_hooks = mod
        try:
            from trn_agent_boot.trn_boot import _ntff_profile_via_ctypes

            mod.set_axon_ntff_profile_hook(
                _ntff_profile_via_ctypes("/opt/axon/libaxon_pjrt.so")
            )
            import concourse.bass_utils as _bu

            _orig_upload = _bu.upload_artifacts

            def _safe_upload(tmpdir):
                try:
                    return _orig_upload(tmpdir)
                except Exception:
                    return f"local:{tmpdir}"

            _bu.upload_artifacts = _safe_upload
        except Exception:
            pass
    except Exception:
        pass


def kernel(x, Wr, W1, b1, W2, b2):
    _ensure_trace_hooks()
    from concourse.bass_utils import run_bass_kernel_spmd

    bf16 = ml_dtypes.bfloat16
    e4 = ml_dtypes.float8_e4m3
    E = NUM_EXPERTS
    B, S, _ = x.shape
    N = B * S
    xm = np.ascontiguousarray(x.reshape(N, D), dtype=np.float32)

    # --- host router (mirrors reference fp32 arithmetic; softmax is
    # monotonic so top-k on probs == top-k on logits, ties broken by index)
    logits = xm @ Wr
    mx = logits.max(axis=1, keepdims=True)
    ex = np.exp(logits - mx)
    probs = ex / ex.sum(axis=1, keepdims=True)
    top_i = np.argsort(-probs, axis=1, kind="stable")[:, :TOP_K]

    idx = [np.where((top_i == e).any(axis=1))[0] for e in range(E)]
    gates = [probs[idx[e], e] for e in range(E)]

    # fp8 threshold: largest gate value such that the fp8-assigned pairs
    # hold at most F8_MASS of the total gate^2 mass
    allg = np.sort(np.concatenate(gates))
    cum = np.cumsum(allg**2) / (allg**2).sum()
    thr = allg[int(np.searchsorted(cum, F8_MASS))]

    # per-expert split: bf16 pairs (g >= thr) first, then fp8 pairs padded
    # to a multiple of 16 (DoubleRow needs the pair stride %16 == 0)
    idx_bf, idx_f8, segs = [], [], []
    for e in range(E):
        m8 = gates[e] < thr
        idx_bf.append(idx[e][~m8])
        idx_f8.append(idx[e][m8])
        n_f8 = int(m8.sum())
        segs.append((int((~m8).sum()), (n_f8 + 15) // 16 * 16))
    segs = tuple(segs)

    # --- dispatch: pack the bf16 and fp8 token streams (identical for
    # every core).  fp8 inner block layout: [KD/2, 2, tbs]
    xT = np.ascontiguousarray(xm.T).astype(bf16)  # [D, N]
    x8T = np.ascontiguousarray(xm.T).astype(e4)
    xbf_e, xf8_e = [], []
    for e in range(E):
        n_bf, n_f8p = segs[e]
        xbf_e.append(xT[:, idx_bf[e]].reshape(KD, P, -1).transpose(1, 0, 2))
        xe8 = np.zeros((D, n_f8p), dtype=e4)
        xe8[:, :len(idx_f8[e])] = x8T[:, idx_f8[e]]
        xf8_e.append(xe8.reshape(KD // 2, 2, P, n_f8p).transpose(2, 0, 1, 3))
    base_bf = np.cumsum([0] + [a for a, _ in segs])
    base_f8 = np.cumsum([0] + [b for _, b in segs])
    sched = _block_schedule(segs)
    blk3 = []  # per-block 3D chunk, in schedule order
    for (e, is_f8, t0, src0, tbs, first) in sched:
        if is_f8:
            s = src0 - base_f8[e]
            blk3.append(xf8_e[e][:, :, :, s:s + tbs])
        else:
            s = src0 - base_bf[e]
            blk3.append(xbf_e[e][:, :, s:s + tbs])
    pairs = _x_pairs(sched)
    partners = set(pairs.values())
    chunks_bf, chunks_f8 = [], []
    for i, (e, is_f8, t0, src0, tbs, first) in enumerate(sched):
        if i in partners:
            continue
        c = blk3[i]
        if i in pairs:  # pack the pair as one [_, ta+tb] unit
            c = np.concatenate([c, blk3[pairs[i]]], axis=-1)
        (chunks_f8 if is_f8 else chunks_bf).append(c.reshape(P, -1))
    xtb = (np.ascontiguousarray(np.concatenate(chunks_bf, axis=1))
           if chunks_bf else np.zeros((P, 1), dtype=bf16))
    xt8 = (np.ascontiguousarray(np.concatenate(chunks_f8, axis=1))
           if chunks_f8 else np.zeros((P, 1), dtype=e4))

    b1f = np.asarray(b1, dtype=np.float32)
    W1f = np.asarray(W1, dtype=np.float32)
    W2f = np.asarray(W2, dtype=np.float32)
    W1b = W1f.astype(bf16)  # [E, D, H]
    W2b = W2f.astype(bf16)  # [E, H, D]
    W18 = (W1f * S1).astype(e4)
    W28 = (W2f * S2).astype(e4)

    in_maps = []
    for c in range(E):
        sl = slice(c * HS, (c + 1) * HS)
        w1c = W1b[:, :, sl].reshape(E, KD, P, HS).transpose(2, 0, 1, 3)
        w2c = W2b[:, sl, :].reshape(E, KH2, P, D).transpose(2, 0, 1, 3)
        w18c = W18[:, :, sl].reshape(E, KD // 2, 2, P, HS).transpose(3, 0, 1, 2, 4)
        w28c = W28[:, sl, :].reshape(E, KH2 // 2, 2, P, D).transpose(3, 0, 1, 2, 4)
        wf8c = np.concatenate(
            [w18c.reshape(P, E, -1), w28c.reshape(P, E, -1)], axis=2)
        b1c = b1f[:, sl].reshape(E, MT, P).transpose(2, 0, 1).reshape(P, E * MT)
        in_maps.append({
            "xtb": xtb,
            "xt8": xt8,
            "w1": np.ascontiguousarray(w1c),
            "w2": np.ascontiguousarray(w2c),
            "wf8": np.ascontiguousarray(wf8c),
            "b1t": np.ascontiguousarray(b1c),
        })

    if segs not in _program_cache:
        _program_cache[segs] = _build_program(segs)
    nc = _program_cache[segs]

    res = run_bass_kernel_spmd(nc, in_maps, core_ids=list(range(E)))

    # --- combine: sum the 8 hidden-slice partials, gate, scatter-add
    Ntot = sum(a + b for a, b in segs)
    accp = np.zeros((P, KD * Ntot), dtype=np.float32)
    for c in range(E):
        accp += np.asarray(res.results[c]["ytr"]).astype(np.float32)
    # unpack block-packed [P, KD*Ntot] into [D, Ntot] (D index = d*128 + p)
    acc = np.empty((KD, P, Ntot), dtype=np.float32)
    for (e, is_f8, t0, src0, tbs, first) in _block_schedule(segs):
        acc[:, :, t0:t0 + tbs] = accp[:, KD * t0:KD * (t0 + tbs)].reshape(
            P, KD, tbs).transpose(1, 0, 2)
    acc = acc.reshape(D, Ntot)

    out = np.zeros((N, D), dtype=np.float32)
    b2f = np.asarray(b2, dtype=np.float32)
    off = 0
    for e in range(E):
        n_bf, n_f8p = segs[e]
        for ids, sc, n_used, width in (
            (idx_bf[e], 1.0, len(idx_bf[e]), n_bf),
            (idx_f8[e], 1.0 / S2, len(idx_f8[e]), n_f8p),
        ):
            if n_used:
                g = probs[ids, e]
                ye = acc[:, off:off + n_used].T * (g[:, None] * sc)
                if b2f[e].any():
                    ye = ye + g[:, None] * b2f[e]
                out[ids] += ye
            off += width
    return out.reshape(B, S, D)


# revision 46
# speedup vs baseline: 1.0038x; 1.0038x over previous
# MoE (8 experts, top-2) on 8 TRN2 NeuronCores — hidden-dim sharded with a
# mixed bf16/fp8 precision schedule.
#
# Every core owns a 512-wide slice of the hidden dim of ALL 8 experts and
# processes ALL token-expert pairs: per-core PE work is independent of
# routing balance (no capacity padding). Host sums the 8 partial outputs.
#
# Precision schedule: pairs whose gate value is below a threshold (chosen so
# the fp8-assigned pairs hold F8_MASS of total gate^2 mass, simulated and
# hw-verified rel err 1.83e-2 < 2e-2 gate) run both matmuls in fp8 e4m3 with DoubleRow
# perf mode: 2 contraction chunks (256) per instruction at the same 216 ns
# as a bf16 128-chunk instruction — measured true 2x. W1 is pre-scaled by
# 32 and W2 by 64 so their sigma~1 values use the e4m3 range; the 1/32 is
# folded into the gelu activation's scale and the 1/64 into the host gate
# multiply.
#
# Emission is software-pipelined one block deep (mm1 of block i+1 before
# mm2 of block i) so the gelu latency of the last h-chunk never stalls PE.
#
# Shapes hardcoded for B=4, S=2048, D=1024, H=4096, E=8; program built per
# routing split and cached on the per-expert (n_bf16, n_fp8) tuple.

import numpy as np
import ml_dtypes

NUM_EXPERTS = 8
TOP_K = 2
P = 128          # SBUF partitions
TB = 512         # token block (matmul moving free size)
D = 1024
H = 4096
HS = H // NUM_EXPERTS   # hidden slice per core (512)
KD = D // P      # mm1 contraction chunks (8) / mm2 output row chunks
MT = HS // P     # mm1 output m-tiles per expert segment (4)
KH2 = HS // P    # mm2 contraction chunks (4)
S1 = 32.0        # fp8 W1 pre-scale
S2 = 64.0        # fp8 W2 pre-scale
F8_MASS = 0.11   # fraction of gate^2 mass allowed onto the fp8 path
                 # (simulated rel err 1.83e-2 vs the 2e-2 gate; the device
                 # error matches the numpy simulation to 4+ digits and the
                 # error model is routing-independent, so this is safe)

_program_cache = {}


def _seg_blocks(n):
    rem = n % TB
    return [TB] * (n // TB) + ([rem] if rem else [])


def _block_schedule(segs):
    # shared by host packing and program builder — layouts must agree.
    # The very first block is carved down to 128 tokens so PE starts as
    # soon as b1 + one W1 m-slice + a small x block have landed.
    blocks = []  # (e, is_f8, t0, src0, tbs, first_of_seg)
    t0 = 0
    src_bf = 0
    src_f8 = 0
    for e in range(len(segs)):
        n_bf, n_f8 = segs[e]
        first = True
        bsizes = _seg_blocks(n_bf)
        if e == 0 and bsizes and bsizes[0] == TB:
            # ramp-up: PE starts after ~1 MB; block 0 is long enough to
            # keep the mm1/ACT pipeline busy while DMA is still ramping
            bsizes = [384, TB - 384] + bsizes[1:]
        for tbs in bsizes:
            blocks.append((e, False, t0, src_bf, tbs, first))
            first = False
            t0 += tbs
            src_bf += tbs
        for tbs in _seg_blocks(n_f8):
            blocks.append((e, True, t0, src_f8, tbs, first))
            first = False
            t0 += tbs
            src_f8 += tbs
    return blocks


def _x_pairs(blocks):
    # fetch two consecutive same-dtype blocks with one DMA; blocks 0/1 stay
    # single for the startup-critical chain.  Shared by host packing and
    # builder: paired blocks are packed as one [_, ta+tb] unit.
    pairs = {}
    j = 2
    while j + 1 < len(blocks):
        if blocks[j][1] == blocks[j + 1][1]:
            pairs[j] = j + 1
            j += 2
        else:
            j += 1
    return pairs


def _build_program(segs, b1_zero=False):
    # segs: tuple of (n_bf, n_f8) per expert; n_f8 multiples of 16
    import concourse.mybir as mybir
    import concourse.tile as tile
    from concourse import bacc

    bf = mybir.dt.bfloat16
    f8 = mybir.dt.float8e4
    f32 = mybir.dt.float32
    Gelu = mybir.ActivationFunctionType.Gelu_apprx_tanh
    DR = mybir.MatmulPerfMode.DoubleRow

    E = NUM_EXPERTS
    Ntot = sum(a + b for a, b in segs)
    Nbf = sum(a for a, _ in segs)
    Nf8 = sum(b for _, b in segs)

    nc = bacc.Bacc(None, target_bir_lowering=False, debug=False)
    xtb = nc.declare_dram_parameter("xtb", [P, max(1, KD * Nbf)], bf, isOutput=False).ap()
    xt8 = nc.declare_dram_parameter("xt8", [P, max(1, KD * Nf8)], f8, isOutput=False).ap()
    w1 = nc.declare_dram_parameter("w1", [P, E, KD, HS], bf, isOutput=False).ap()
    w2 = nc.declare_dram_parameter("w2", [P, E, KH2, D], bf, isOutput=False).ap()
    # w18 (KD/2*2*HS = 4096) and w28 (KH2/2*2*D = 4096) packed per expert
    wf8 = nc.declare_dram_parameter("wf8", [P, E, 8192], f8, isOutput=False).ap()
    b1t = nc.declare_dram_parameter("b1t", [P, E * MT], f32, isOutput=False).ap()
    # block-packed output: block at t0 occupies [KD*t0, KD*(t0+tbs)) per
    # partition, contiguous — 8x fewer DMA descriptor runs than [D, Ntot]
    ytr = nc.declare_dram_parameter("ytr", [P, KD * Ntot], bf, isOutput=True).ap()

    # block schedule: per expert, bf16 blocks then fp8 blocks
    blocks = _block_schedule(segs)

    with tile.TileContext(nc) as tc:
        with (
            tc.tile_pool(name="w1p", bufs=2) as w1p,
            tc.tile_pool(name="w2p", bufs=2) as w2p,
            tc.tile_pool(name="w18p", bufs=2) as w18p,
            tc.tile_pool(name="w28p", bufs=2) as w28p,
            tc.tile_pool(name="cst", bufs=1) as cst,
            tc.tile_pool(name="xin", bufs=3) as xpool,
            tc.tile_pool(name="hbuf", bufs=2) as hpool,
            tc.tile_pool(name="yout", bufs=3) as ypool,
            tc.tile_pool(name="ph", bufs=4, space="PSUM") as php,
            tc.tile_pool(name="py", bufs=4, space="PSUM") as pyp,
        ):
            b1_sb = cst.tile([P, E * MT], f32, tag="b1sb")
            if b1_zero:
                # spec fills b1 with zeros: memset beats a startup-chain DMA
                nc.vector.memset(b1_sb, 0.0)
            else:
                nc.sync.dma_start(b1_sb, b1t)

            w1_sb = [None] * E
            w2_sb = [None] * E
            w18_sb = [None] * E
            w28_sb = [None] * E

            def fetch_w1(e, only_m0=False, skip_m0=False):
                # expert 0 is fetched as 4 m-slice tiles so the first matmul
                # group only waits for 256 KB (startup-critical); later
                # experts fetch one contiguous 1 MB DMA (fewer descriptors)
                if only_m0 or skip_m0:
                    if not skip_m0:
                        w1_sb[e] = []
                    for m in range(MT):
                        if (only_m0 and m > 0) or (skip_m0 and m == 0):
                            continue
                        t = w1p.tile([P, KD, P], bf, tag=f"w1sb{m}",
                                     name=f"w1sb{e}_{m}")
                        nc.sync.dma_start(t, w1[:, e, :, m * P:(m + 1) * P])
                        w1_sb[e].append(t)
                else:
                    t = w1p.tile([P, KD, HS], bf, tag="w1full",
                                 name=f"w1sb{e}")
                    nc.sync.dma_start(t, w1[:, e, :, :])
                    w1_sb[e] = t

            def fetch_w2(e):
                w2_sb[e] = w2p.tile([P, KH2, D], bf, tag="w2sb", name=f"w2sb{e}")
                nc.sync.dma_start(w2_sb[e], w2[:, e, :, :])

            def fetch_f8w(e):
                t = w18p.tile([P, 8192], f8, tag="wf8sb", name=f"wf8sb{e}")
                nc.sync.dma_start(t, wf8[:, e, :])
                w18_sb[e] = t[:, :4096].rearrange(
                    "p (k j m) -> p k j m", k=KD // 2, j=2)
                w28_sb[e] = t[:, 4096:].rearrange(
                    "p (k j m) -> p k j m", k=KH2 // 2, j=2)

            fetch_w1(0, only_m0=True)  # startup chain: b1 + w1[0]m0 + x0

            # fetch plan: expert e's weights stream during segment e-1,
            # spread one ~1 MB fetch per block so the x stream is never
            # starved behind a weight burst (x starvation stalls PE and
            # triggers HAM clock-throttle).  Fetches are emitted after that
            # iteration's pending mm2 (pool-buffer WAR ordering).
            seg_start = {}
            for j, bj in enumerate(blocks):
                seg_start.setdefault(bj[0], j)
            fetch_plan = {}

            def plan(i, kind, fe):
                fetch_plan.setdefault(i, []).append((kind, fe))

            plan(0, "w2", 0)  # needed by mm2(block 0), emitted at i==1
            plan(2, "f8", 0)
            for fe in range(1, E):
                anchor = seg_start[fe - 1] + (3 if fe == 1 else 0)
                last = seg_start[fe] - 1  # stay within segment fe-1
                plan(min(anchor, last), "w1", fe)
                plan(min(anchor + 1, last), "w2", fe)
                plan(min(anchor + 2, last), "f8", fe)

            x_pair_next = _x_pairs(blocks)
            x_ap = {}  # block index -> AP for its x data

            def emit_x_single(blk):
                e, is_f8, t0, src0, tbs, first = blk
                if is_f8:
                    xt_blk = xpool.tile([P, KD // 2, 2, tbs], f8, tag="xt8")
                    nc.sync.dma_start(
                        xt_blk,
                        xt8[:, KD * src0:KD * (src0 + tbs)].rearrange(
                            "p (k j c) -> p k j c", k=KD // 2, j=2
                        ),
                    )
                else:
                    xt_blk = xpool.tile([P, KD, tbs], bf, tag="xtb")
                    nc.sync.dma_start(
                        xt_blk,
                        xtb[:, KD * src0:KD * (src0 + tbs)].rearrange(
                            "p (k c) -> p k c", k=KD
                        ),
                    )
                return xt_blk

            def emit_x(i):
                if i in x_ap:
                    return x_ap.pop(i)
                jj = x_pair_next.get(i)
                if jj is None:
                    return emit_x_single(blocks[i])
                blk_a, blk_b = blocks[i], blocks[jj]
                ta, tb = blk_a[4], blk_b[4]
                is_f8 = blk_a[1]
                src0 = blk_a[3]
                if is_f8:
                    tile_ = xpool.tile([P, KD // 2, 2, ta + tb], f8, tag="xt8")
                    nc.sync.dma_start(
                        tile_,
                        xt8[:, KD * src0:KD * (src0 + ta + tb)].rearrange(
                            "p (k j c) -> p k j c", k=KD // 2, j=2
                        ),
                    )
                    x_ap[jj] = tile_[:, :, :, ta:]
                    return tile_[:, :, :, :ta]
                tile_ = xpool.tile([P, KD, ta + tb], bf, tag="xtb")
                nc.sync.dma_start(
                    tile_,
                    xtb[:, KD * src0:KD * (src0 + ta + tb)].rearrange(
                        "p (k c) -> p k c", k=KD
                    ),
                )
                x_ap[jj] = tile_[:, :, ta:]
                return tile_[:, :, :ta]

            def emit_mm1(blk, xt_blk):
                e, is_f8, t0, src0, tbs, first = blk
                if is_f8:
                    hT = hpool.tile([P, KH2, tbs], f8, tag="hT8")
                    for m in range(MT):
                        ph = php.tile([P, tbs], f32, tag="ph")
                        for k in range(KD // 2):
                            nc.tensor.matmul(
                                ph,
                                w18_sb[e][:, k, :, m * P:(m + 1) * P],
                                xt_blk[:, k, :, :],
                                start=(k == 0),
                                stop=(k == KD // 2 - 1),
                                perf_mode=DR,
                            )
                        nc.scalar.activation(
                            hT[:, m, :], ph, Gelu,
                            bias=b1_sb[:, e * MT + m:e * MT + m + 1],
                            scale=1.0 / S1,
                        )
                else:
                    hT = hpool.tile([P, KH2, tbs], bf, tag="hT")
                    for m in range(MT):
                        ph = php.tile([P, tbs], f32, tag="ph")
                        w1e = w1_sb[e]
                        for k in range(KD):
                            nc.tensor.matmul(
                                ph,
                                w1e[m][:, k, :] if isinstance(w1e, list)
                                else w1e[:, k, m * P:(m + 1) * P],
                                xt_blk[:, k, :],
                                start=(k == 0),
                                stop=(k == KD - 1),
                            )
                        nc.scalar.activation(
                            hT[:, m, :], ph, Gelu,
                            bias=b1_sb[:, e * MT + m:e * MT + m + 1],
                        )
                return hT

            def emit_mm2(blk, hT, pair=None):
                # pair=(tile, off, total, flush): even blocks allocate a
                # 2-block tile; odd blocks reuse it and flush one DMA
                e, is_f8, t0, src0, tbs, first = blk
                if pair is None:
                    yt_all = ypool.tile([P, KD * tbs], bf, tag="yt")
                    off, total, flush, dma_t0 = 0, KD * tbs, True, t0
                else:
                    yt_all, off, total, flush, dma_t0 = pair
                for dd in range(KD):
                    py = pyp.tile([P, tbs], f32, tag="py")
                    if is_f8:
                        for k in range(KH2 // 2):
                            nc.tensor.matmul(
                                py,
                                w28_sb[e][:, k, :, dd * P:(dd + 1) * P],
                                hT[:, 2 * k:2 * k + 2, :],
                                start=(k == 0),
                                stop=(k == KH2 // 2 - 1),
                                perf_mode=DR,
                            )
                    else:
                        for k in range(KH2):
                            nc.tensor.matmul(
                                py,
                                w2_sb[e][:, k, dd * P:(dd + 1) * P],
                                hT[:, k, :],
                                start=(k == 0),
                                stop=(k == KH2 - 1),
                            )
                    nc.vector.tensor_copy(
                        yt_all[:, off + dd * tbs:off + (dd + 1) * tbs], py)
                if flush:
                    nc.sync.dma_start(
                        ytr[:, KD * dma_t0:KD * dma_t0 + total],
                        yt_all[:, :total],
                    )

            pending = None  # (blk, hT) awaiting mm2
            x_next = None  # x(block 1), primed ahead of the w2[0] fetch
            ypair = None   # open 2-block y tile: (tile, off, total, flush, t0)
            nmm2 = 0       # mm2 emission count (pairing parity)

            def emit_mm2_paired(blk, hT):
                nonlocal ypair, nmm2
                tbs = blk[4]
                if ypair is None:
                    if nmm2 + 1 < len(blocks):
                        nxt = blocks[nmm2 + 1][4]
                        tile_ = ypool.tile([P, KD * (tbs + nxt)], bf, tag="yt")
                        emit_mm2(blk, hT,
                                 pair=(tile_, 0, KD * tbs, False, blk[2]))
                        ypair = (tile_, KD * tbs, KD * (tbs + nxt), True,
                                 blk[2])
                    else:
                        emit_mm2(blk, hT)  # unpaired tail block
                else:
                    emit_mm2(blk, hT, pair=ypair)
                    ypair = None
                nmm2 += 1

            for i, blk in enumerate(blocks):
                e, is_f8, t0, src0, tbs, first = blk
                xt_blk = x_next if x_next is not None else emit_x(i)
                x_next = None
                if i == 0:
                    fetch_w1(0, skip_m0=True)  # behind x0 in the queue
                hT = emit_mm1(blk, xt_blk)
                if i == 0 and len(blocks) > 1:
                    x_next = emit_x(1)
                # weight streaming, ordered behind the x blocks that are
                # needed sooner (DMAs drain roughly in emission order):
                # i==1: w2[0]; i==2: fp8 weights[0]; i==3: all of seg 1;
                # then one segment ahead at each later segment boundary.
                # Startup fetches use fresh pool buffers so they may precede
                # the pending mm2; the segment-boundary fetch reuses a buffer
                # whose last reader is the pending mm2, so it must follow it.
                if pending is not None:
                    emit_mm2_paired(*pending)
                for kind, fe in fetch_plan.get(i, ()):
                    (fetch_w1 if kind == "w1" else
                     fetch_w2 if kind == "w2" else fetch_f8w)(fe)
                pending = (blk, hT)
            emit_mm2_paired(*pending)
    nc.compile()
    return nc


def _ensure_trace_hooks():
    # bass_utils' trace path (taken when BASS_TRACE=1 is set externally)
    # imports antenv.axon_hooks, which this image lacks. Shim it (and the
    # artifact upload, which needs a bucket) only when missing, so tracing
    # degrades gracefully instead of crashing.
    import sys
    import types

    try:
        import antenv.axon_hooks  # noqa: F401
        return
    except ImportError:
        pass
    try:
        import antenv

        mod = types.ModuleType("antenv.axon_hooks")
        state = {"hook": None}
        mod.set_axon_ntff_profile_hook = lambda h: state.__setitem__("hook", h)
        mod.get_axon_ntff_profile_hook = lambda: state["hook"]
        sys.modules["antenv.axon_hooks"] = mod
        antenv.axon_hooks = mod
        try:
            from trn_agent_boot.trn_boot import _ntff_profile_via_ctypes

            mod.set_axon_ntff_profile_hook(
                _ntff_profile_via_ctypes("/opt/axon/libaxon_pjrt.so")
            )
            import concourse.bass_utils as _bu

            _orig_upload = _bu.upload_artifacts

            def _safe_upload(tmpdir):
                try:
                    return _orig_upload(tmpdir)
                except Exception:
                    return f"local:{tmpdir}"

            _bu.upload_artifacts = _safe_upload
        except Exception:
            pass
    except Exception:
        pass


def kernel(x, Wr, W1, b1, W2, b2):
    _ensure_trace_hooks()
    from concourse.bass_utils import run_bass_kernel_spmd

    bf16 = ml_dtypes.bfloat16
    e4 = ml_dtypes.float8_e4m3
    E = NUM_EXPERTS
    B, S, _ = x.shape
    N = B * S
    xm = np.ascontiguousarray(x.reshape(N, D), dtype=np.float32)

    # --- host router (mirrors reference fp32 arithmetic; softmax is
    # monotonic so top-k on probs == top-k on logits, ties broken by index)
    logits = xm @ Wr
    mx = logits.max(axis=1, keepdims=True)
    ex = np.exp(logits - mx)
    probs = ex / ex.sum(axis=1, keepdims=True)
    top_i = np.argsort(-probs, axis=1, kind="stable")[:, :TOP_K]

    idx = [np.where((top_i == e).any(axis=1))[0] for e in range(E)]
    gates = [probs[idx[e], e] for e in range(E)]

    # fp8 threshold: largest gate value such that the fp8-assigned pairs
    # hold at most F8_MASS of the total gate^2 mass
    allg = np.sort(np.concatenate(gates))
    cum = np.cumsum(allg**2) / (allg**2).sum()
    thr = allg[int(np.searchsorted(cum, F8_MASS))]

    # per-expert split: bf16 pairs (g >= thr) first, then fp8 pairs padded
    # to a multiple of 16 (DoubleRow needs the pair stride %16 == 0)
    idx_bf, idx_f8, segs = [], [], []
    for e in range(E):
        m8 = gates[e] < thr
        idx_bf.append(idx[e][~m8])
        idx_f8.append(idx[e][m8])
        n_f8 = int(m8.sum())
        segs.append((int((~m8).sum()), (n_f8 + 15) // 16 * 16))
    segs = tuple(segs)

    # --- dispatch: pack the bf16 and fp8 token streams (identical for
    # every core).  fp8 inner block layout: [KD/2, 2, tbs]
    xT = np.ascontiguousarray(xm.T).astype(bf16)  # [D, N]
    x8T = np.ascontiguousarray(xm.T).astype(e4)
    xbf_e, xf8_e = [], []
    for e in range(E):
        n_bf, n_f8p = segs[e]
        xbf_e.append(xT[:, idx_bf[e]].reshape(KD, P, -1).transpose(1, 0, 2))
        xe8 = np.zeros((D, n_f8p), dtype=e4)
        xe8[:, :len(idx_f8[e])] = x8T[:, idx_f8[e]]
        xf8_e.append(xe8.reshape(KD // 2, 2, P, n_f8p).transpose(2, 0, 1, 3))
    base_bf = np.cumsum([0] + [a for a, _ in segs])
    base_f8 = np.cumsum([0] + [b for _, b in segs])
    sched = _block_schedule(segs)
    blk3 = []  # per-block 3D chunk, in schedule order
    for (e, is_f8, t0, src0, tbs, first) in sched:
        if is_f8:
            s = src0 - base_f8[e]
            blk3.append(xf8_e[e][:, :, :, s:s + tbs])
        else:
            s = src0 - base_bf[e]
            blk3.append(xbf_e[e][:, :, s:s + tbs])
    pairs = _x_pairs(sched)
    partners = set(pairs.values())
    chunks_bf, chunks_f8 = [], []
    for i, (e, is_f8, t0, src0, tbs, first) in enumerate(sched):
        if i in partners:
            continue
        c = blk3[i]
        if i in pairs:  # pack the pair as one [_, ta+tb] unit
            c = np.concatenate([c, blk3[pairs[i]]], axis=-1)
        (chunks_f8 if is_f8 else chunks_bf).append(c.reshape(P, -1))
    xtb = (np.ascontiguousarray(np.concatenate(chunks_bf, axis=1))
           if chunks_bf else np.zeros((P, 1), dtype=bf16))
    xt8 = (np.ascontiguousarray(np.concatenate(chunks_f8, axis=1))
           if chunks_f8 else np.zeros((P, 1), dtype=e4))

    b1f = np.asarray(b1, dtype=np.float32)
    W1f = np.asarray(W1, dtype=np.float32)
    W2f = np.asarray(W2, dtype=np.float32)
    W1b = W1f.astype(bf16)  # [E, D, H]
    W2b = W2f.astype(bf16)  # [E, H, D]
    W18 = (W1f * S1).astype(e4)
    W28 = (W2f * S2).astype(e4)

    in_maps = []
    for c in range(E):
        sl = slice(c * HS, (c + 1) * HS)
        w1c = W1b[:, :, sl].reshape(E, KD, P, HS).transpose(2, 0, 1, 3)
        w2c = W2b[:, sl, :].reshape(E, KH2, P, D).transpose(2, 0, 1, 3)
        w18c = W18[:, :, sl].reshape(E, KD // 2, 2, P, HS).transpose(3, 0, 1, 2, 4)
        w28c = W28[:, sl, :].reshape(E, KH2 // 2, 2, P, D).transpose(3, 0, 1, 2, 4)
        wf8c = np.concatenate(
            [w18c.reshape(P, E, -1), w28c.reshape(P, E, -1)], axis=2)
        b1c = b1f[:, sl].reshape(E, MT, P).transpose(2, 0, 1).reshape(P, E * MT)
        in_maps.append({
            "xtb": xtb,
            "xt8": xt8,
            "w1": np.ascontiguousarray(w1c),
            "w2": np.ascontiguousarray(w2c),
            "wf8": np.ascontiguousarray(wf8c),
            "b1t": np.ascontiguousarray(b1c),
        })

    b1_zero = not b1f.any()
    key = (segs, b1_zero)
    if key not in _program_cache:
        _program_cache[key] = _build_program(segs, b1_zero)
    nc = _program_cache[key]

    res = run_bass_kernel_spmd(nc, in_maps, core_ids=list(range(E)))

    # --- combine: sum the 8 hidden-slice partials, gate, scatter-add
    Ntot = sum(a + b for a, b in segs)
    accp = np.zeros((P, KD * Ntot), dtype=np.float32)
    for c in range(E):
        accp += np.asarray(res.results[c]["ytr"]).astype(np.float32)
    # unpack block-packed [P, KD*Ntot] into [D, Ntot] (D index = d*128 + p)
    acc = np.empty((KD, P, Ntot), dtype=np.float32)
    for (e, is_f8, t0, src0, tbs, first) in _block_schedule(segs):
        acc[:, :, t0:t0 + tbs] = accp[:, KD * t0:KD * (t0 + tbs)].reshape(
            P, KD, tbs).transpose(1, 0, 2)
    acc = acc.reshape(D, Ntot)

    out = np.zeros((N, D), dtype=np.float32)
    b2f = np.asarray(b2, dtype=np.float32)
    off = 0
    for e in range(E):
        n_bf, n_f8p = segs[e]
        for ids, sc, n_used, width in (
            (idx_bf[e], 1.0, len(idx_bf[e]), n_bf),
            (idx_f8[e], 1.0 / S2, len(idx_f8[e]), n_f8p),
        ):
            if n_used:
                g = probs[ids, e]
                ye = acc[:, off:off + n_used].T * (g[:, None] * sc)
                if b2f[e].any():
                    ye = ye + g[:, None] * b2f[e]
                out[ids] += ye
            off += width
    return out.reshape(B, S, D)
